# revision 10
# baseline (speedup 1.0000x reference)
"""Self-contained 8-core Trainium2 Bass kernel for nn_MultiHeadAttention.

Problem: x:[4,2048,1024] f32, w_qkv:[3072,1024], b_qkv:[3072],
w_proj:[1024,1024], b_proj:[1024].  16 heads, head_dim 64.

Sharding: core c = batch(4) x head-group(2).  Each core computes QKV for
its 8 heads on its batch, attention, and a partial output projection over
its 512 head-dims.  Host sums the two partials per batch and adds b_proj.

v4 design (per-core dataflow, all matmuls bf16, fp32 PSUM):
  - Energy matmuls for a head PAIR are row-tiled: head 2p uses PE rows
    0:63 (K=64 contraction over d), head 2p+1 rows 64:127 -> the two mms
    execute concurrently.
  - exp work is split between the ACT engine (true Exp LUT, scale=16)
    and a custom DVE op EXP16 computing (1+c1*t+c2*t^2)^16 ~= exp(16 t)
    (max rel err ~3e-3 on |E|<=2.6).  The softmax scale folded into the
    q weights is 1/(8*16) so PSUM energies arrive as E/16.
  - att@V: one K=128 M=65 mm per head per kt (65th ones-column in v makes
    psum row 64 the softmax denominator), lag-2 behind exp.
  - block-end accumulator drains are deferred into the next block's slots
    (post_block generator) so they don't head-block the in-order ACT/DVE
    queues at boundaries.
  - denominators: psum row 64 -> DRAM bounce -> packed [32,512] -> one
    DVE approx-reciprocal -> DRAM -> broadcast-DMA builds bc -> one DVE
    scalar_tensor_tensor per (pair, qg) normalizes outT in SBUF.
  - qk projection for pair p+1 (PSUM pq pool, 2 bufs -- single-buffering
    head-blocks the PE queue) + normalization of pair p-1 are interleaved
    into pair p's attention slots as PE/DVE fillers.
"""
import sys

sys.path.insert(0, "/opt/trn_rl_repo")

import numpy as np
import ml_dtypes

import concourse.bass as bass
import concourse.mybir as mybir
import concourse.tile as tile
from concourse import bacc
from concourse.bass_utils import run_bass_kernel_spmd

bf16 = ml_dtypes.bfloat16
F32 = mybir.dt.float32
BF16 = mybir.dt.bfloat16

B, N, EMB = 4, 2048, 1024
HEADS, HD = 16, 64
HPC = 8            # heads per core
KAUG = 1152        # 1024 emb + 1 bias row, padded to 9*128
NKT = KAUG // 128  # 9 contraction tiles for v
NT_N = N // 128    # 16 n-tiles
EXPF = mybir.ActivationFunctionType.Exp

# exp(x) ~= (1 + c1*(x/16) + c2*(x/16)^2)^16, fitted on |x| <= 2.6
EXP_C1 = 1.003224
EXP_C2 = 0.499913
# kt slots (of 16) whose exp runs on the DVE custom op instead of ACT
DVE_KT = (3, 7, 11, 15)


def _register_exp16():
    """Register the custom DVE op (idempotent)."""
    from concourse import dve_ops
    from concourse.dve_ops import DveOp, OPS
    from concourse.dve_spec import Spec, Src0, C0, C1, One, lower, sq
    from concourse.dve_uop import DveOpSpec

    name = "EXP16_ANT"
    if name in dve_ops._SUB_OPCODE_FOR_NAME:
        return next(op for op in OPS if op.name == name)
    body = sq(sq(sq(sq(One + Src0 * (C0 + Src0 * C1)))))

    def ref(in0, in1, c0, c1, c2):
        q = (1.0 + in0 * (c0 + in0 * c1)).astype(np.float32)
        for _ in range(4):
            q = (q * q).astype(np.float32)
        return q

    spec = Spec(body=body, reference=ref)
    row = dve_ops._CUSTOM_DVE_ROW_BASE + len(OPS)
    dve_ops._SUB_OPCODE_FOR_NAME[name] = row
    shas = {}
    for ver in ("v3", "v4"):
        s = DveOpSpec(name=name, opcode=row, uops=lower(spec, ver=ver),
                      rd1_en=False)
        shas[ver] = s.sha(ver)
    op = DveOp(name, spec, subdim=False, uops_sha=shas)
    OPS.append(op)
    dve_ops.CUSTOM_DVE_SPECS[name] = spec
    return op


EXP16 = _register_exp16()


def _build_kernel(ctx, tc, nc, xT, wqk, wv, wp, y, loop_k=1):
    """loop_k>1 wraps the whole body in an on-device For_i (timing NEFFs)."""
    if loop_k > 1:
        with tc.For_i(0, loop_k):
            _build_kernel(ctx, tc, nc, xT, wqk, wv, wp, y, loop_k=1)
        return
    mult = mybir.AluOpType.mult
    add = mybir.AluOpType.add

    const = ctx.enter_context(tc.tile_pool(name="const", bufs=1))
    qkp = ctx.enter_context(tc.tile_pool(name="qkp", bufs=1))
    vp = ctx.enter_context(tc.tile_pool(name="vp", bufs=1))
    outp = ctx.enter_context(tc.tile_pool(name="outp", bufs=1))
    attp = ctx.enter_context(tc.tile_pool(name="attp", bufs=4))
    misc = ctx.enter_context(tc.tile_pool(name="misc", bufs=1))
    stage = ctx.enter_context(tc.tile_pool(name="stage", bufs=3))
    bcp = ctx.enter_context(tc.tile_pool(name="bcp", bufs=3))
    yp = ctx.enter_context(tc.tile_pool(name="yp", bufs=3))
    pe = ctx.enter_context(tc.tile_pool(name="pe", bufs=2, space="PSUM"))
    pq = ctx.enter_context(tc.tile_pool(name="pq", bufs=2, space="PSUM"))
    po = ctx.enter_context(tc.tile_pool(name="po", bufs=1, space="PSUM"))
    dramp = ctx.enter_context(tc.tile_pool(name="dramp", bufs=1, space="DRAM"))

    # ---- load inputs: xT+wv first (v-phase deps), wqk next, wp last ----
    wv_t = []
    for kt in range(NKT):
        t = const.tile([128, 512], BF16, tag=f"wv{kt}", name=f"wv{kt}")
        nc.gpsimd.dma_start(t[:], wv[kt * 128:(kt + 1) * 128, :])
        wv_t.append(t)
    xT_t = []
    for kt in range(NKT):
        t = const.tile([128, N], BF16, tag=f"xT{kt}", name=f"xT{kt}")
        (nc.sync if kt % 2 == 0 else nc.scalar).dma_start(
            t[:], xT[kt * 128:(kt + 1) * 128, :])
        xT_t.append(t)
    # qk bias vectors (tiny, needed by the first qk bias-adds)
    bq_t = []
    for t8 in range(8):
        tb = misc.tile([128, 1], BF16, tag=f"bqb{t8}", name=f"bqb{t8}")
        nc.gpsimd.dma_start(tb[:], wqk[EMB:EMB + 1, t8 * 128:(t8 + 1) * 128])
        t = misc.tile([128, 1], F32, tag=f"bq{t8}", name=f"bq{t8}")
        nc.vector.tensor_copy(t[:], tb[:])
        bq_t.append(t)
    wqk_t = []
    for kt in range(8):  # only EMB rows: qk bias added separately
        t = const.tile([128, 1024], BF16, tag=f"wqk{kt}", name=f"wqk{kt}")
        nc.gpsimd.dma_start(t[:], wqk[kt * 128:(kt + 1) * 128, :])
        wqk_t.append(t)
    wp_t = []
    for t4 in range(4):
        t = const.tile([128, 1024], BF16, tag=f"wp{t4}", name=f"wp{t4}")
        nc.gpsimd.dma_start(t[:], wp[t4 * 128:(t4 + 1) * 128, :])
        wp_t.append(t)

    # qk m-tiles 0..3 = q of head pairs (h%2 on partition halves), 4..7 = k.
    qkT = [qkp.tile([128, N], BF16, tag=f"qkT{h}", name=f"qkT{h}") for h in range(HPC)]
    v_t = [vp.tile([128, HPC, 65], BF16, tag=f"v{nt}", name=f"v{nt}") for nt in range(NT_N)]
    outT_raw = [outp.tile([128, N], BF16, tag=f"or{t}", name=f"or{t}") for t in range(4)]
    outT_n = [outp.tile([128, N], BF16, tag=f"on{t}", name=f"on{t}") for t in range(4)]
    den_dram = dramp.tile([32, 512], F32, name="den_dram")
    rec_dram = dramp.tile([32, 512], F32, name="rec_dram")
    den_pk = misc.tile([32, 512], F32, tag="den", name="den_pk")
    rec_pk = misc.tile([32, 512], F32, tag="rec", name="rec_pk")
    nc.vector.memset(den_pk[:], 1.0)

    def emit_v_tile(nt):
        p = pe.tile([128, 1024], F32, tag="pe", name="pep")
        for kt in range(NKT):
            nc.tensor.matmul(
                p[:, 0:512],
                xT_t[kt][:, nt * 128:(nt + 1) * 128],
                wv_t[kt][:],
                start=(kt == 0), stop=(kt == NKT - 1),
            )
        nc.scalar.copy(
            v_t[nt][:, :, 0:64],
            p[:, 0:512].rearrange("p (h c) -> p h c", c=64),
        )
        nc.vector.memset(v_t[nt][:, :, 64:65], 1.0)

    def qk_pair_prologue(hp):
        """qk projection for pair hp using pe-pool [128,1024] tiles."""
        for t in (hp, 4 + hp):
            for nbp in range(2):
                p = pe.tile([128, 1024], F32, tag="pe", name="pep")
                for j in range(2):
                    for kt in range(8):
                        nc.tensor.matmul(
                            p[:, j * 512:(j + 1) * 512],
                            wqk_t[kt][:, t * 128:(t + 1) * 128],
                            xT_t[kt][:, nbp * 1024 + j * 512:
                                     nbp * 1024 + (j + 1) * 512],
                            start=(kt == 0), stop=(kt == 7),
                        )
                nc.vector.tensor_scalar_add(
                    qkT[t][:, nbp * 1024:(nbp + 1) * 1024], p[:], bq_t[t][:])

    def qk_pair_gen(hp):
        """Generator: one PE mm (or DVE bias-add) per step, [128,512] groups
        from the 1-bank pq pool."""
        for t in (hp, 4 + hp):
            for nbp in range(2):
                for j in range(2):
                    p = pq.tile([128, 512], F32, tag="pq", name="pqt")
                    for kt in range(8):
                        nc.tensor.matmul(
                            p[:],
                            wqk_t[kt][:, t * 128:(t + 1) * 128],
                            xT_t[kt][:, nbp * 1024 + j * 512:
                                     nbp * 1024 + (j + 1) * 512],
                            start=(kt == 0), stop=(kt == 7),
                        )
                        yield
                    nc.vector.tensor_scalar_add(
                        qkT[t][:, nbp * 1024 + j * 512:
                               nbp * 1024 + (j + 1) * 512],
                        p[:], bq_t[t][:])
                    yield

    def den_recip(nrows):
        nc.gpsimd.dma_start(den_pk[0:nrows, :], den_dram[0:nrows, :])
        nc.vector.reciprocal_approx_fast(rec_pk[:], den_pk[:])
        nc.gpsimd.dma_start(rec_dram[0:nrows, :], rec_pk[0:nrows, :])

    def norm_step(p_, qg):
        """bc built by broadcast-DMA from rec_dram; one DVE stt normalizes
        both heads' rows.  Yields between ops."""
        rA = (2 * p_) * 4 + qg
        rB = (2 * p_ + 1) * 4 + qg
        bc = bcp.tile([128, 512], F32, tag="bc", name="bc")
        nc.sync.dma_start(bc[0:64, :], rec_dram[rA, :].partition_broadcast(64))
        nc.sync.dma_start(bc[64:128, :], rec_dram[rB, :].partition_broadcast(64))
        yield
        nc.vector.scalar_tensor_tensor(
            outT_n[p_][:, qg * 512:(qg + 1) * 512],
            outT_raw[p_][:, qg * 512:(qg + 1) * 512],
            1.0,
            bc[:],
            op0=mult, op1=mult,
        )
        yield

    def normalize_gen(p_):
        for qg in range(4):
            yield from norm_step(p_, qg)

    def attv_mms(p_, kt, at_t, accA, accB):
        hA, hB = 2 * p_, 2 * p_ + 1
        nc.tensor.matmul(
            accA[0:65, :], v_t[kt][:, hA, :], at_t[:, 0:512],
            start=(kt == 0), stop=(kt == NT_N - 1))
        nc.tensor.matmul(
            accB[0:65, :], v_t[kt][:, hB, :], at_t[:, 512:1024],
            start=(kt == 0), stop=(kt == NT_N - 1))

    def proj_nt(nt):
        p = pe.tile([128, 1024], F32, tag="pe", name="pep")
        for ng in range(2):
            for t4 in range(4):
                nc.tensor.matmul(
                    p[:, ng * 512:(ng + 1) * 512],
                    outT_n[t4][:, nt * 128:(nt + 1) * 128],
                    wp_t[t4][:, ng * 512:(ng + 1) * 512],
                    start=(t4 == 0), stop=(t4 == 3),
                )
        ys = yp.tile([128, 1024], F32, tag="y", name="ys")
        nc.vector.tensor_copy(ys[:, 0:512], p[:, 0:512])
        nc.scalar.copy(ys[:, 512:1024], p[:, 512:1024])
        nc.sync.dma_start(y[nt * 128:(nt + 1) * 128, :], ys[:])

    # ---- prologue: v tiles + qk pair 0 (PE-only, exp engines idle) ----
    for nt in range(NT_N):
        emit_v_tile(nt)
        if nt == 6:
            qk_pair_prologue(0)

    def post_block(p_, qc, accA, accB):
        """Block-end accumulator drains, deferred into the next block's
        slots so they don't head-block the exp engines at the boundary."""
        rA = (2 * p_) * 4 + qc
        rB = (2 * p_ + 1) * 4 + qc
        stA = stage.tile([128, 512], F32, tag="st", name="st")
        nc.scalar.copy(stA[64:65, :], accA[64:65, :])
        nc.gpsimd.dma_start(den_dram[rA:rA + 1, :], stA[64:65, :])
        yield
        stB = stage.tile([128, 512], F32, tag="st", name="st")
        nc.scalar.copy(stB[64:65, :], accB[64:65, :])
        nc.gpsimd.dma_start(den_dram[rB:rB + 1, :], stB[64:65, :])
        yield
        nc.vector.tensor_copy(
            outT_raw[p_][0:64, qc * 512:(qc + 1) * 512], accA[0:64, :])
        yield
        nc.vector.tensor_copy(
            outT_raw[p_][64:128, qc * 512:(qc + 1) * 512], accB[0:64, :])
        yield

    # ---- attention over 4 head pairs ----
    filler = iter(())      # PE filler: next pair's qk mms
    dve_filler = iter(())  # DVE filler: previous pair's den recip + normalize
    pfiller = iter(())     # block-end drains of the previous block
    for p_ in range(4):
        if p_ + 1 < 4:
            for _ in filler:
                pass
            filler = qk_pair_gen(p_ + 1)
        for qc in range(4):
            accA = po.tile([65, 512], F32, tag="poA", name="poA")
            accB = po.tile([65, 512], F32, tag="poB", name="poB")
            ats = []
            for kt in range(NT_N):
                pet = pe.tile([128, 1024], F32, tag="pe", name="pep")
                nc.tensor.matmul(
                    pet[:, 0:512],
                    qkT[4 + p_][0:64, kt * 128:(kt + 1) * 128],
                    qkT[p_][0:64, qc * 512:(qc + 1) * 512],
                    start=True, stop=True,
                )
                nc.tensor.matmul(
                    pet[:, 512:1024],
                    qkT[4 + p_][64:128, kt * 128:(kt + 1) * 128],
                    qkT[p_][64:128, qc * 512:(qc + 1) * 512],
                    start=True, stop=True,
                )
                at_t = attp.tile([128, 1024], BF16, tag="att", name="at")
                if kt in DVE_KT:
                    nc.vector._custom_dve(EXP16, out=at_t[:], in0=pet[:],
                                          s0=EXP_C1, s1=EXP_C2)
                else:
                    nc.scalar.activation(at_t[:], pet[:], EXPF, scale=16.0)
                ats.append(at_t)
                if kt >= 2:
                    attv_mms(p_, kt - 2, ats[kt - 2], accA, accB)
                next(pfiller, None)
                next(filler, None)
                if kt % 3 == 2:
                    next(dve_filler, None)
            attv_mms(p_, NT_N - 2, ats[NT_N - 2], accA, accB)
            attv_mms(p_, NT_N - 1, ats[NT_N - 1], accA, accB)
            pfiller = post_block(p_, qc, accA, accB)
        # pair p_ done: drain the last block's post ops, then queue den
        # reciprocal + normalization into the next pair's slots.
        for _ in pfiller:
            pass
        pfiller = iter(())
        if p_ < 3:
            for _ in dve_filler:
                pass
            def _pair_tail(p_=p_):
                den_recip((p_ + 1) * 8)
                yield
                yield from normalize_gen(p_)
            dve_filler = _pair_tail()
    for _ in filler:
        pass
    for _ in dve_filler:
        pass

    # ---- tail: normalize pair 3 per q-chunk, proj interleaved ----
    den_recip(32)
    for qg in range(4):
        for _ in norm_step(3, qg):
            pass
        for nt in range(qg * 4, qg * 4 + 4):
            proj_nt(nt)


_CACHE = {}


def _get_nc(loop_k=1):
    key = f"nc{loop_k}"
    if key not in _CACHE:
        nc = bacc.Bacc("TRN2", target_bir_lowering=False, debug=False, num_devices=8)
        xT = nc.dram_tensor("xT", [KAUG, N], BF16, kind="ExternalInput")
        wqk = nc.dram_tensor("wqk", [KAUG, 1024], BF16, kind="ExternalInput")
        wv = nc.dram_tensor("wv", [KAUG, 512], BF16, kind="ExternalInput")
        wp = nc.dram_tensor("wp", [512, 1024], BF16, kind="ExternalInput")
        y = nc.dram_tensor("y", [N, EMB], F32, kind="ExternalOutput")
        with tile.TileContext(nc) as tc:
            from contextlib import ExitStack
            with ExitStack() as es:
                _build_kernel(es, tc, nc, xT.ap(), wqk.ap(), wv.ap(), wp.ap(),
                              y.ap(), loop_k=loop_k)
        nc.compile()
        _CACHE[key] = nc
    return _CACHE[key]


def make_in_maps(x, w_qkv, b_qkv, w_proj):
    """Host-side shard prep: per-core bf16 operands with folded biases/scale.

    The softmax scale folded into q is 1/(8*16): the extra 1/16 is undone by
    the exp stage (ACT scale=16 / DVE x^16 polynomial)."""
    x = np.asarray(x, np.float32)
    w_qkv = np.asarray(w_qkv, np.float32)
    b_qkv = np.asarray(b_qkv, np.float32)
    w_proj = np.asarray(w_proj, np.float32)
    scale = 1.0 / (np.sqrt(HD) * 16.0)

    in_maps = []
    for c in range(8):
        b, g = divmod(c, 2)
        heads = range(g * HPC, (g + 1) * HPC)

        xT_aug = np.zeros((KAUG, N), np.float32)
        xT_aug[0:EMB, :] = x[b].T
        xT_aug[EMB, :] = 1.0

        wqk = np.zeros((KAUG, 1024), np.float32)
        wv = np.zeros((KAUG, 512), np.float32)
        for hl, H in enumerate(heads):
            qs, ks, vs = H * HD, EMB + H * HD, 2 * EMB + H * HD
            # q cols: m-tile hl//2, partition half hl%2; k cols: m-tile 4+hl//2
            qc = (hl // 2) * 128 + (hl % 2) * 64
            kc = 512 + qc
            wqk[0:EMB, qc:qc + 64] = w_qkv[qs:qs + HD, :].T * scale
            wqk[EMB, qc:qc + 64] = b_qkv[qs:qs + HD] * scale
            wqk[0:EMB, kc:kc + 64] = w_qkv[ks:ks + HD, :].T
            wqk[EMB, kc:kc + 64] = b_qkv[ks:ks + HD]
            wv[0:EMB, hl * 64:(hl + 1) * 64] = w_qkv[vs:vs + HD, :].T
            wv[EMB, hl * 64:(hl + 1) * 64] = b_qkv[vs:vs + HD]

        wp = w_proj[:, g * 512:(g + 1) * 512].T.copy()

        in_maps.append({
            "xT": xT_aug.astype(bf16),
            "wqk": wqk.astype(bf16),
            "wv": wv.astype(bf16),
            "wp": wp.astype(bf16),
        })
    return in_maps


def kernel(x, w_qkv, b_qkv, w_proj, b_proj):
    x = np.asarray(x, np.float32)
    b_proj = np.asarray(b_proj, np.float32)
    nc = _get_nc()
    in_maps = make_in_maps(x, w_qkv, b_qkv, w_proj)
    res = run_bass_kernel_spmd(nc, in_maps, core_ids=list(range(8)))
    out = np.empty((B, N, EMB), np.float32)
    for b in range(B):
        out[b] = res.results[2 * b]["y"] + res.results[2 * b + 1]["y"] + b_proj
    return out
